# revision 3
# baseline (speedup 1.0000x reference)
"""MinDistanceDecoder (vq_codebook) Trainium2 kernel.

Math: reference computes, per batch row b,
    argmin_w mean_n |llr[b,n] - max_abs * s[w,n]|,   s[w,n] in {+1,-1}
with llr = -4*noisy/sigma2 and max_abs = max|llr|.  Since |llr| <= max_abs
elementwise, |llr - max_abs*s| = max_abs - s*llr exactly, so
    dist[b,w] = max_abs - (1/n) * sum_n s[w,n]*llr[b,n]
and argmin_w dist = argmax_w sum_n s[w,n]*llr[b,n] = argmin_w noisy[b]·s[w]
(llr is a negative scalar multiple of noisy; sigma2 > 0).  The returned value
is possible_words[best] = the LSB-first bit pattern of the argmin index.

Kernel: codebook-sharded across 8 cores (8192 codewords each).  Each core
computes scores'[b,w] = (-noisy[b])·s[w] via bf16 hi/lo matmuls (exact to
~1e-5, verified >> top-2 gap), then finds per-1024-chunk top-8 maxima and
their indices with the DVE Max/MaxIndex instructions.  Host combines the
8 cores x 8 chunks candidates and decodes bits.
"""

import numpy as np
import ml_dtypes

K = 16
N = 32
B = 64
NW = 2 ** K            # 65536
NCORES = 8
WPC = NW // NCORES     # 8192 codewords per core
HALF = WPC // 2        # 4096  (w-halves stacked on partitions 0-63 / 64-127)
TS = 512               # psum tile columns
NPAIR = HALF // TS     # 8
CHUNK = 1024           # DVE argmax chunk columns
NCHUNK = HALF // CHUNK  # 4

_CACHE = {}


def _split_excess_waits(nc, mybir, maxw=1):
    """Walrus (core_v3) rejects instructions carrying >2 sem waits ("Too many
    sync wait commands") -- split extras onto standalone event-sem waits."""
    for f in nc.m.functions:
        for bb in f.blocks:
            new = []
            for ins in bb.instructions:
                si = ins.sync_info
                if si is not None and si.on_wait and len(si.on_wait) > maxw:
                    waits = list(si.on_wait)
                    extra, keep = waits[:-maxw], waits[-maxw:]
                    for j, w in enumerate(extra):
                        sw = mybir.InstEventSemaphore(
                            name=f"{ins.name}-wsplit{j}", ins=[], outs=[],
                            sync_info=mybir.SyncInfo(on_wait=[w], on_update=[]))
                        sw.engine = ins.engine
                        new.append(sw)
                    ins.sync_info = mybir.SyncInfo(
                        on_wait=keep, on_update=list(si.on_update))
                new.append(ins)
            bb.instructions = new


def _build(split_waits=True):
    import concourse.bass as bass
    import concourse.mybir as mybir
    from concourse.tile import TileContext

    nc = bass.Bass()
    sT = nc.dram_tensor("sT", [N, WPC], mybir.dt.bfloat16, kind="ExternalInput")
    xhi = nc.dram_tensor("xhi", [N, B], mybir.dt.bfloat16, kind="ExternalInput")
    xlo = nc.dram_tensor("xlo", [N, B], mybir.dt.bfloat16, kind="ExternalInput")
    vals = nc.dram_tensor("vals", [128, 8 * NCHUNK], mybir.dt.float32,
                          kind="ExternalOutput")
    idxs = nc.dram_tensor("idxs", [128, 8 * NCHUNK], mybir.dt.uint32,
                          kind="ExternalOutput")

    with TileContext(nc) as tc:
        with (
            tc.tile_pool(name="inp", bufs=1) as inp_pool,
            tc.tile_pool(name="cb", bufs=1) as cb_pool,
            tc.tile_pool(name="ps", bufs=4, space="PSUM") as psum_pool,
            tc.tile_pool(name="sc", bufs=1) as sc_pool,
            tc.tile_pool(name="o", bufs=1) as out_pool,
        ):
            xh = inp_pool.tile([N, B], mybir.dt.bfloat16, tag="xh")
            xl = inp_pool.tile([N, B], mybir.dt.bfloat16, tag="xl")
            nc.sync.dma_start(xh[:], xhi[:])
            nc.sync.dma_start(xl[:], xlo[:])

            st = cb_pool.tile([N, WPC], mybir.dt.bfloat16)
            # 4 column-chunk DMAs so matmuls can start before the whole
            # codebook lands.
            dchunk = WPC // 4
            for d in range(4):
                cs = slice(d * dchunk, (d + 1) * dchunk)
                nc.sync.dma_start(st[:, cs], sT[:, cs])

            scores = sc_pool.tile([128, HALF], mybir.dt.float32)
            v8 = out_pool.tile([128, 8 * NCHUNK], mybir.dt.float32, tag="v8")
            i8 = out_pool.tile([128, 8 * NCHUNK], mybir.dt.uint32, tag="i8")

            for t in range(NPAIR):
                ps = psum_pool.tile([128, TS], mybir.dt.float32)
                ca = slice(t * TS, (t + 1) * TS)
                cb = slice(HALF + t * TS, HALF + (t + 1) * TS)
                nc.tensor.matmul(ps[0:64, :], xh[:], st[:, ca],
                                 start=True, stop=False)
                nc.tensor.matmul(ps[0:64, :], xl[:], st[:, ca],
                                 start=False, stop=True)
                nc.tensor.matmul(ps[64:128, :], xh[:], st[:, cb],
                                 start=True, stop=False)
                nc.tensor.matmul(ps[64:128, :], xl[:], st[:, cb],
                                 start=False, stop=True)
                nc.scalar.copy(scores[:, ca], ps[:])

            for k in range(NCHUNK):
                ch = scores[:, k * CHUNK:(k + 1) * CHUNK]
                vsl = v8[:, k * 8:(k + 1) * 8]
                nc.vector.max(out=vsl, in_=ch)
                nc.vector.max_index(out=i8[:, k * 8:(k + 1) * 8],
                                    in_max=vsl, in_values=ch)

            nc.sync.dma_start(vals[:], v8[:])
            nc.sync.dma_start(idxs[:], i8[:])

    if split_waits:
        _split_excess_waits(nc, mybir)
    return nc


def _get_nc():
    if "nc" not in _CACHE:
        _CACHE["nc"] = _build()
    return _CACHE["nc"]


def _host_codebook_sT(G):
    """sT[n, w] = 1 - 2*((words[w] @ G) % 2), bf16, [N, NW]."""
    Gb = (np.asarray(G) % 2).astype(np.uint8)
    w_idx = np.arange(NW, dtype=np.uint32)
    bits = ((w_idx[:, None] >> np.arange(K)[None, :]) & 1).astype(np.uint8)
    cw = np.zeros((NW, N), dtype=np.uint8)
    for i in range(K):
        np.bitwise_xor(cw, bits[:, i:i + 1] & Gb[i][None, :], out=cw)
    s = (1.0 - 2.0 * cw.astype(np.float32))
    return np.ascontiguousarray(s.T).astype(ml_dtypes.bfloat16), bits


def kernel(noisy_symbols, G, sigma2):
    from concourse.bass_utils import run_bass_kernel_spmd

    noisy = np.asarray(noisy_symbols, dtype=np.float32)
    assert noisy.shape == (B, N)

    # scores' = (-noisy) @ s^T ; maximize.  sigma2 > 0 only scales.
    xT = np.ascontiguousarray((-noisy).T)            # [N, B] f32
    xh32 = xT.astype(ml_dtypes.bfloat16).astype(np.float32)
    xhi = xh32.astype(ml_dtypes.bfloat16)
    xlo = (xT - xh32).astype(ml_dtypes.bfloat16)

    sT_full, bits = _host_codebook_sT(G)             # [N, NW] bf16

    in_maps = []
    for c in range(NCORES):
        in_maps.append({
            "sT": np.ascontiguousarray(sT_full[:, c * WPC:(c + 1) * WPC]),
            "xhi": xhi,
            "xlo": xlo,
        })

    nc = _get_nc()
    res = run_bass_kernel_spmd(nc, in_maps, list(range(NCORES)))
    _CACHE["last_results"] = res

    # Host combine: per core/chunk top-1 value + index -> global argmax
    # (ties -> smallest codeword index, matching jnp.argmin first-index).
    best_val = np.full((B,), -np.inf, dtype=np.float64)
    best_w = np.zeros((B,), dtype=np.int64)
    p = np.arange(128)
    b_of_p = p % 64
    h_of_p = p // 64
    for c in range(NCORES):
        v = np.asarray(res.results[c]["vals"])       # [128, 8*NCHUNK] f32
        ix = np.asarray(res.results[c]["idxs"])      # [128, 8*NCHUNK] u32
        for k in range(NCHUNK):
            val = v[:, k * 8].astype(np.float64)     # top-1 of chunk
            j = ix[:, k * 8].astype(np.int64)
            w = c * WPC + h_of_p * HALF + k * CHUNK + j
            for pp in range(128):
                bb = b_of_p[pp]
                if (val[pp] > best_val[bb]) or (
                        val[pp] == best_val[bb] and w[pp] < best_w[bb]):
                    best_val[bb] = val[pp]
                    best_w[bb] = w[pp]

    return bits[best_w].astype(np.float32)           # [B, K] LSB-first bits


# revision 15
# speedup vs baseline: 1.0916x; 1.0916x over previous
"""MinDistanceDecoder (vq_codebook) Trainium2 kernel.

Math: reference computes, per batch row b,
    argmin_w mean_n |llr[b,n] - max_abs * s[w,n]|,   s[w,n] in {+1,-1}
with llr = -4*noisy/sigma2 and max_abs = max|llr|.  Since |llr| <= max_abs
elementwise, |llr - max_abs*s| = max_abs - s*llr exactly, so
    dist[b,w] = max_abs - (1/n) * sum_n s[w,n]*llr[b,n]
and argmin_w dist = argmax_w sum_n s[w,n]*llr[b,n] = argmin_w noisy[b]·s[w]
(llr is a negative scalar multiple of noisy; sigma2 > 0).  The returned value
is possible_words[best] = the LSB-first bit pattern of the argmin index.

Kernel: codebook-sharded across 8 cores (8192 codewords each).  Each core
computes scores'[b,w] = (-noisy[b])·s[w] via bf16 hi/lo matmuls (exact to
~1e-5, verified >> top-2 score gap of 1.7e-2), stacking two codeword halves
on PSUM partitions 0-63 / 64-127, then finds per-1024-column-chunk top-8
maxima and their first-occurrence indices with the DVE Max8/FindIndex8
instructions.  Host combines 8 cores x 4 chunks x 2 halves candidates and
decodes the winning index into its bit pattern.

Layout: pair t (t=0..7) scores sT columns [1024t, 1024t+512) on partitions
0-63 and [1024t+512, 1024(t+1)) on partitions 64-127, so one contiguous
2048-column DMA chunk feeds exactly two pairs; codebook DMA is split into 4
such chunks alternating between the two HWDGE engines (ACT pushes chunk 0
first) so the first matmul starts as early as possible.
"""

import numpy as np
import ml_dtypes

K = 16
N = 32
B = 64
NW = 2 ** K            # 65536
NCORES = 8
WPC = NW // NCORES     # 8192 codewords per core
HALF = WPC // 2        # 4096 scores columns (x2 partition halves)
TS = 512               # psum tile columns
NPAIR = HALF // TS     # 8
# DVE argmax chunks (columns of the scores tile); smaller tail chunks so the
# final Max8/FindIndex8 pair costs less after the last PSUM copy lands.
DVE_CHUNKS = [512, 1024, 1024, 1024, 512]
DVE_BASES = [0, 512, 1536, 2560, 3584]
NCHUNK = len(DVE_CHUNKS)
# codebook DMA column chunks (leading ones small for an early PE start)
DMA_CHUNKS = [1024, 1024, 2048, 2048, 2048]
DMA_BASES = [0, 1024, 2048, 4096, 6144]

_CACHE = {}


def _split_excess_waits(nc, mybir, maxw=1):
    """Walrus (core_v3) rejects instructions carrying too many sem waits
    ("Too many sync wait commands") -- split extras onto standalone
    event-semaphore wait instructions placed just before."""
    for f in nc.m.functions:
        for bb in f.blocks:
            new = []
            for ins in bb.instructions:
                si = ins.sync_info
                if si is not None and si.on_wait and len(si.on_wait) > maxw:
                    waits = list(si.on_wait)
                    extra, keep = waits[:-maxw], waits[-maxw:]
                    for j, w in enumerate(extra):
                        sw = mybir.InstEventSemaphore(
                            name=f"{ins.name}-wsplit{j}", ins=[], outs=[],
                            sync_info=mybir.SyncInfo(on_wait=[w], on_update=[]))
                        sw.engine = ins.engine
                        new.append(sw)
                    ins.sync_info = mybir.SyncInfo(
                        on_wait=keep, on_update=list(si.on_update))
                new.append(ins)
            bb.instructions = new


def _build(split_waits=True):
    import concourse.bass as bass
    import concourse.mybir as mybir
    from concourse.tile import TileContext

    nc = bass.Bass()
    sT = nc.dram_tensor("sT", [N, WPC], mybir.dt.float8e4, kind="ExternalInput")
    x2 = nc.dram_tensor("x2", [N, 2 * B], mybir.dt.bfloat16,
                        kind="ExternalInput")   # cols 0-63 hi, 64-127 lo
    out = nc.dram_tensor("out", [128, 16 * NCHUNK], mybir.dt.uint32,
                         kind="ExternalOutput")  # per chunk: 8 val (f32 bits) + 8 idx

    with TileContext(nc) as tc:
        with (
            tc.tile_pool(name="inp", bufs=1) as inp_pool,
            tc.tile_pool(name="cb", bufs=1) as cb_pool,
            tc.tile_pool(name="ps", bufs=4, space="PSUM") as psum_pool,
            tc.tile_pool(name="sc", bufs=1) as sc_pool,
            tc.tile_pool(name="o", bufs=1) as out_pool,
        ):
            xt = inp_pool.tile([N, 2 * B], mybir.dt.bfloat16)
            nc.sync.dma_start(xt[:], x2[:])
            xh = xt[:, 0:B]
            xl = xt[:, B:2 * B]

            st = cb_pool.tile([N, WPC], mybir.dt.float8e4)
            # Codebook (fp8: +/-1 is exact) split over both HWDGE queues;
            # ACT pushes chunk 0 while sync starts with x2.
            dma_engines = [nc.scalar, nc.sync, nc.scalar, nc.sync, nc.scalar]
            for d in range(5):
                cs = slice(DMA_BASES[d], DMA_BASES[d] + DMA_CHUNKS[d])
                dma_engines[d].dma_start(st[:, cs], sT[:, cs])

            scores = sc_pool.tile([128, HALF], mybir.dt.float32)
            ot = out_pool.tile([128, 16 * NCHUNK], mybir.dt.uint32)

            for t in range(NPAIR):
                ps = psum_pool.tile([128, TS], mybir.dt.float32)
                ca = slice(1024 * t, 1024 * t + 512)         # half A columns
                cb = slice(1024 * t + 512, 1024 * (t + 1))   # half B columns
                nc.tensor.matmul(ps[0:64, :], xh, st[:, ca],
                                 start=True, stop=False)
                nc.tensor.matmul(ps[0:64, :], xl, st[:, ca],
                                 start=False, stop=True)
                nc.tensor.matmul(ps[64:128, :], xh, st[:, cb],
                                 start=True, stop=False)
                nc.tensor.matmul(ps[64:128, :], xl, st[:, cb],
                                 start=False, stop=True)
                sc_cols = slice(t * TS, (t + 1) * TS)
                nc.scalar.copy(scores[:, sc_cols], ps[:])

            for k in range(NCHUNK):
                ch = scores[:, DVE_BASES[k]:DVE_BASES[k] + DVE_CHUNKS[k]]
                vsl = ot[:, 16 * k:16 * k + 8].bitcast(mybir.dt.float32)
                nc.vector.max(out=vsl, in_=ch)
                nc.vector.max_index(out=ot[:, 16 * k + 8:16 * k + 16],
                                    in_max=vsl, in_values=ch)

            nc.sync.dma_start(out[:], ot[:])

    if split_waits:
        _split_excess_waits(nc, mybir)
    return nc


def _get_nc():
    if "nc" not in _CACHE:
        _CACHE["nc"] = _build()
    return _CACHE["nc"]


def _host_codebook_sT(G):
    """sT[n, w] = 1 - 2*((words[w] @ G) % 2), fp8e4m3, [N, NW]."""
    Gb = (np.asarray(G) % 2).astype(np.uint8)
    w_idx = np.arange(NW, dtype=np.uint32)
    bits = ((w_idx[:, None] >> np.arange(K)[None, :]) & 1).astype(np.uint8)
    cw = np.zeros((NW, N), dtype=np.uint8)
    for i in range(K):
        np.bitwise_xor(cw, bits[:, i:i + 1] & Gb[i][None, :], out=cw)
    s = (1.0 - 2.0 * cw.astype(np.float32))
    return np.ascontiguousarray(s.T).astype(ml_dtypes.float8_e4m3), bits


def kernel(noisy_symbols, G, sigma2):
    from concourse.bass_utils import run_bass_kernel_spmd

    noisy = np.asarray(noisy_symbols, dtype=np.float32)
    assert noisy.shape == (B, N)

    # scores' = (-noisy) @ s^T ; maximize.  sigma2 > 0 only scales.
    xT = np.ascontiguousarray((-noisy).T)            # [N, B] f32
    xh32 = xT.astype(ml_dtypes.bfloat16).astype(np.float32)
    x2 = np.concatenate(
        [xh32.astype(ml_dtypes.bfloat16),
         (xT - xh32).astype(ml_dtypes.bfloat16)], axis=1)   # [N, 2B]
    x2 = np.ascontiguousarray(x2)

    sT_full, bits = _host_codebook_sT(G)             # [N, NW] bf16

    in_maps = []
    for c in range(NCORES):
        in_maps.append({
            "sT": np.ascontiguousarray(sT_full[:, c * WPC:(c + 1) * WPC]),
            "x2": x2,
        })

    nc = _get_nc()
    res = run_bass_kernel_spmd(nc, in_maps, list(range(NCORES)))
    _CACHE["last_results"] = res

    # Host combine: per (core, chunk, half) top-1 value + index -> global
    # argmax (ties -> smallest codeword index, matching jnp.argmin).
    # Column c of the scores tile maps to w_local = 1024*(c//512) + 512*h
    # + (c%512); FindIndex8 returns the first occurrence, which is the
    # smallest w_local within a chunk for fixed h.
    best_val = np.full((B,), -np.inf, dtype=np.float64)
    best_w = np.zeros((B,), dtype=np.int64)
    p = np.arange(128)
    b_of_p = p % 64
    h_of_p = p // 64
    for c in range(NCORES):
        o = np.asarray(res.results[c]["out"])        # [128, 16*NCHUNK] u32
        for k in range(NCHUNK):
            val = np.ascontiguousarray(o[:, 16 * k]).view(np.float32).astype(np.float64)
            col = DVE_BASES[k] + o[:, 16 * k + 8].astype(np.int64)
            w = c * WPC + 1024 * (col // 512) + 512 * h_of_p + (col % 512)
            for pp in range(128):
                bb = b_of_p[pp]
                if (val[pp] > best_val[bb]) or (
                        val[pp] == best_val[bb] and w[pp] < best_w[bb]):
                    best_val[bb] = val[pp]
                    best_w[bb] = w[pp]

    return bits[best_w].astype(np.float32)           # [B, K] LSB-first bits
